# revision 8
# baseline (speedup 1.0000x reference)
"""Trainium2 Bass kernel for nn_EncoderLayer_54116587929733.

Data-parallel over batch: B=8 batches -> 8 NeuronCores, no collectives.

Per-core math (batch b, S=2048, d=256, H=8 heads, hid=2048):
  The reference computes, per head h:
    scores = A^T K_h^T / sqrt(d)            [d, S]
    attn   = softmax_S(scores)              [d, S]   (output!)
    M      = attn @ V_h                     [d, d]
    QA     = Q_h @ A                        [S, d]
    ctx    = QA @ M^T                       [S, d]
  then concat heads, out-proj + 3 LayerNorms + FFN.

  We fold A into the weights on the host (weights are compile-time
  constants in deployment; this is 0.2% of total FLOPs):
    WKP_h = Wk_h @ A / sqrt(d)   => scores   = WKP_h^T @ x^T   (K never materialized)
    WQP_h = Wq_h @ A             => QA^T     = WQP_h^T @ x^T   (Q never materialized)
  Softmax denominators are folded into the ctx evacuation (row scale),
  so the attn@V contraction runs on unnormalized exp(scores^T).

  All matmuls run in float32r (full fp32 operand width, 1 cycle/row on
  the PE for moving dim >= 256, ~1.6e-4 rel err vs 2.6e-3 for bf16).
"""

import sys

sys.path.insert(0, "/opt/trn_rl_repo")

import numpy as np

S = 2048
D = 256
H = 8
B = 8
HID = 2048
EPS = 1e-5
NST = S // 128      # 16 s-tiles
NCT = D // 128      # 2 c/e/dl-tiles
NCH = 4             # s-chunks of 512
CH = S // NCH       # 512

_CACHE = {}


def _split_multi_waits(nc, mybir):
    """This walrus build supports only ONE sync-wait per instruction;
    Tile attaches several.  Move extras onto same-engine NOPs inserted
    just before the offending instruction."""
    cnt = 0
    for fn in nc.m.functions:
        for blk in fn.blocks:
            new_insts = []
            for inst in blk.instructions:
                si = inst.sync_info
                if si is not None and si.on_wait and len(si.on_wait) > 1:
                    waits = list(si.on_wait)
                    for w in waits[:-1]:
                        cnt += 1
                        new_insts.append(mybir.InstNoOp(
                            name=f"{inst.name}_wsplit{cnt}", ins=[], outs=[],
                            engine=inst.engine,
                            sync_info=mybir.SyncInfo(on_wait=[w], on_update=[])))
                    si.on_wait = waits[-1:]
                new_insts.append(inst)
            blk.instructions = new_insts
    return cnt


def _build():
    import concourse.bass as bass
    import concourse.mybir as mybir
    import concourse.tile as tile
    from concourse.masks import make_identity

    f32 = mybir.dt.float32
    f32r = mybir.dt.float32r
    AF = mybir.ActivationFunctionType

    nc = bass.Bass(target_bir_lowering=False)

    # ---- DRAM I/O --------------------------------------------------
    x_d = nc.dram_tensor("x", [S, D], f32, kind="ExternalInput")
    xT_d = nc.dram_tensor("xT", [D, S], f32r, kind="ExternalInput")
    wkp_d = nc.dram_tensor("wkp", [D, HID], f32r, kind="ExternalInput")
    wqp_d = nc.dram_tensor("wqp", [D, HID], f32r, kind="ExternalInput")
    wv_d = nc.dram_tensor("wv", [D, HID], f32r, kind="ExternalInput")
    wo_d = nc.dram_tensor("wo", [HID, D], f32r, kind="ExternalInput")
    w1_d = nc.dram_tensor("w1", [D, HID], f32r, kind="ExternalInput")
    w2_d = nc.dram_tensor("w2", [HID, D], f32r, kind="ExternalInput")
    b1t_d = nc.dram_tensor("b1t", [128, NST], f32, kind="ExternalInput")
    # broadcast-to-128-partitions [128, D] host-prepped vectors
    bvec_names = ["b2b", "g1b", "be1b", "g2b", "be2b", "g3b", "be3b"]
    bvec_d = {n: nc.dram_tensor(n, [128, D], f32, kind="ExternalInput")
              for n in bvec_names}

    attn_d = nc.dram_tensor("attn_o", [H, D, S], f32, kind="ExternalOutput")
    out_d = nc.dram_tensor("out_o", [S, D], f32, kind="ExternalOutput")

    with tile.TileContext(nc) as tc:
        import contextlib
        ctx = contextlib.ExitStack()
        with ctx:
            persist = ctx.enter_context(tc.tile_pool(name="persist", bufs=1))

            # persistent loads
            xT_s = persist.tile([128, NCT, S], f32r)
            nc.sync.dma_start(out=xT_s[:], in_=xT_d.rearrange("(t p) s -> p t s", p=128))
            x_s = persist.tile([128, NST, D], f32)
            nc.sync.dma_start(out=x_s[:], in_=x_d.rearrange("(t p) c -> p t c", p=128))
            wqp_s = persist.tile([128, NCT, HID], f32r)
            nc.sync.dma_start(out=wqp_s[:], in_=wqp_d.rearrange("(t p) n -> p t n", p=128))
            wo_s = persist.tile([128, NST, D], f32r)
            nc.sync.dma_start(out=wo_s[:], in_=wo_d.rearrange("(t p) c -> p t c", p=128))
            w1_s = persist.tile([128, NCT, HID], f32r)
            nc.sync.dma_start(out=w1_s[:], in_=w1_d.rearrange("(t p) n -> p t n", p=128))
            w2_s = persist.tile([128, NST, D], f32r)
            nc.sync.dma_start(out=w2_s[:], in_=w2_d.rearrange("(t p) c -> p t c", p=128))
            b1t_s = persist.tile([128, NST], f32)
            nc.sync.dma_start(out=b1t_s[:], in_=b1t_d[:])
            bvec_s = {}
            for n in bvec_names:
                bvec_s[n] = persist.tile([128, D], f32, name=f"bv_{n}", tag=f"bv_{n}")
                nc.sync.dma_start(out=bvec_s[n][:], in_=bvec_d[n][:])

            eps_s = persist.tile([128, 1], f32)
            nc.vector.memset(eps_s, EPS)
            ones_f = persist.tile([128, 1], f32)
            nc.vector.memset(ones_f, 1.0)
            ones_s = persist.tile([128, 1], f32r)
            nc.vector.tensor_copy(out=ones_s[:], in_=ones_f[:])
            ident_s = persist.tile([128, 128], f32)
            make_identity(nc, ident_s)

            # A->B carriers
            mut_s = persist.tile([128, NCT, H, D], f32r)     # [j, jt, h, dl]
            recip_s = persist.tile([128, H, NCT], f32)       # [dl, h, dlt]

            # ---------------- Phase A: per-head K/V side ----------------
            with (
                tc.tile_pool(name="poolA", bufs=1) as poolA,
                tc.tile_pool(name="psA", bufs=1, space="PSUM") as psA,
            ):
                wkp_s = poolA.tile([128, NCT, HID], f32r, tag="wkp")
                nc.sync.dma_start(out=wkp_s[:], in_=wkp_d.rearrange("(t p) n -> p t n", p=128))
                wv_s = poolA.tile([128, NCT, HID], f32r, tag="wv")
                nc.sync.dma_start(out=wv_s[:], in_=wv_d.rearrange("(t p) n -> p t n", p=128))

                for h in range(H):
                    hs = h * D
                    # V_h natural layout [s, j]
                    v_s = poolA.tile([128, NST, D], f32r, tag="v", bufs=1)
                    for st in range(NST):
                        pv = psA.tile([128, D], f32, tag="pv", bufs=1)
                        for kt in range(NCT):
                            nc.tensor.matmul(
                                pv[:], xT_s[:, kt, st * 128:(st + 1) * 128],
                                wv_s[:, kt, hs:hs + D],
                                start=(kt == 0), stop=(kt == NCT - 1))
                        nc.vector.tensor_copy(out=v_s[:, st, :], in_=pv[:])

                    # scores [e, s] -> E = exp(scores) (f32, for attn output)
                    e_s = poolA.tile([128, NCT, S], f32, tag="e", bufs=1)
                    for et in range(NCT):
                        for chk in range(NCH):
                            sc = psA.tile([128, CH], f32, tag="sc", bufs=2)
                            for kt in range(NCT):
                                nc.tensor.matmul(
                                    sc[:], wkp_s[:, kt, hs + et * 128:hs + (et + 1) * 128],
                                    xT_s[:, kt, chk * CH:(chk + 1) * CH],
                                    start=(kt == 0), stop=(kt == NCT - 1))
                            nc.scalar.activation(
                                out=e_s[:, et, chk * CH:(chk + 1) * CH], in_=sc[:],
                                func=AF.Exp)

                    # scores^T [s, e] -> ETu = exp (f32r, matmul operand)
                    etu_s = poolA.tile([128, NST, D], f32r, tag="etu", bufs=1)
                    for st in range(NST):
                        stp = psA.tile([128, D], f32, tag="stp", bufs=2)
                        for kt in range(NCT):
                            nc.tensor.matmul(
                                stp[:], xT_s[:, kt, st * 128:(st + 1) * 128],
                                wkp_s[:, kt, hs:hs + D],
                                start=(kt == 0), stop=(kt == NCT - 1))
                        nc.scalar.activation(out=etu_s[:, st, :], in_=stp[:], func=AF.Exp)

                    # denominators: row [1, 256] = sum_s ETu[s, :]
                    pd = psA.tile([1, D], f32, tag="pd", bufs=1)
                    for st in range(NST):
                        nc.tensor.matmul(pd[:], ones_s[:, :1], etu_s[:, st, :],
                                         start=(st == 0), stop=(st == NST - 1))
                    denr = poolA.tile([1, D], f32, tag="denr", bufs=1)
                    nc.vector.tensor_copy(out=denr[:], in_=pd[:])
                    # transpose to column + reciprocal: [dl-part, 1]
                    # (plain f32 matmul: K=1 f32r hits an ISA restriction)
                    for et in range(NCT):
                        pdt = psA.tile([128, 1], f32, tag="pdt", bufs=1)
                        nc.tensor.matmul(pdt[:], denr[:1, et * 128:(et + 1) * 128],
                                         ones_f[:1, :1], start=True, stop=True)
                        nc.vector.reciprocal(out=recip_s[:, h, et:et + 1], in_=pdt[:])

                    # attn output: normalize rows of E and DMA out
                    for et in range(NCT):
                        nc.vector.tensor_scalar_mul(
                            out=e_s[:, et, :], in0=e_s[:, et, :],
                            scalar1=recip_s[:, h, et:et + 1])
                        nc.sync.dma_start(
                            out=attn_d[h, et * 128:(et + 1) * 128, :],
                            in_=e_s[:, et, :])

                    # MuT [j, dl] = V^T @ ETu (unnormalized)
                    for jt in range(NCT):
                        pmu = psA.tile([128, D], f32, tag="pmu", bufs=1)
                        for st in range(NST):
                            nc.tensor.matmul(
                                pmu[:], v_s[:, st, jt * 128:(jt + 1) * 128],
                                etu_s[:, st, :],
                                start=(st == 0), stop=(st == NST - 1))
                        nc.vector.tensor_copy(out=mut_s[:, jt, h, :], in_=pmu[:])

            # ---------------- Phase B: Q side + out-proj + FFN ----------------
            def ln_block(pool, src_f32_ap, gname, bname, dst_ap):
                """dst = LN(src) * g + b   (src: [128, D] f32 SBUF)"""
                stats = pool.tile([128, 6], f32, tag="lnstats", bufs=4)
                nc.vector.bn_stats(out=stats[:], in_=src_f32_ap)
                mv = pool.tile([128, 2], f32, tag="lnmv", bufs=4)
                nc.vector.bn_aggr(out=mv[:], in_=stats[:])
                nc.scalar.activation(out=mv[:, 1:2], in_=mv[:, 1:2],
                                     func=AF.Sqrt, bias=eps_s[:], scale=1.0)
                nc.vector.reciprocal(out=mv[:, 1:2], in_=mv[:, 1:2])
                nc.vector.tensor_scalar(
                    out=dst_ap, in0=src_f32_ap,
                    scalar1=mv[:, 0:1], scalar2=mv[:, 1:2],
                    op0=mybir.AluOpType.subtract, op1=mybir.AluOpType.mult)
                nc.vector.tensor_mul(out=dst_ap, in0=dst_ap, in1=bvec_s[gname][:])
                nc.vector.tensor_add(out=dst_ap, in0=dst_ap, in1=bvec_s[bname][:])

            with (
                tc.tile_pool(name="poolB", bufs=1) as poolB,
                tc.tile_pool(name="psB", bufs=1, space="PSUM") as psB,
            ):
                for chk in range(NCH):
                    c0 = chk * CH
                    # per-head QA^T and ctx^T for this s-chunk
                    ctx_s = poolB.tile([128, NST, CH], f32r, tag="ctx", bufs=1)
                    for h in range(H):
                        hs = h * D
                        qat = poolB.tile([128, NCT, CH], f32r, tag="qat", bufs=1)
                        for et in range(NCT):
                            pqa = psB.tile([128, CH], f32, tag="pqa", bufs=2)
                            for kt in range(NCT):
                                nc.tensor.matmul(
                                    pqa[:], wqp_s[:, kt, hs + et * 128:hs + (et + 1) * 128],
                                    xT_s[:, kt, c0:c0 + CH],
                                    start=(kt == 0), stop=(kt == NCT - 1))
                            nc.scalar.activation(out=qat[:, et, :], in_=pqa[:],
                                                 func=AF.Copy)
                        for dt_ in range(NCT):
                            pct = psB.tile([128, CH], f32, tag="pct", bufs=1)
                            for et in range(NCT):
                                nc.tensor.matmul(
                                    pct[:], mut_s[:, et, h, dt_ * 128:(dt_ + 1) * 128],
                                    qat[:, et, :],
                                    start=(et == 0), stop=(et == NCT - 1))
                            nc.vector.tensor_scalar_mul(
                                out=ctx_s[:, h * NCT + dt_, :], in0=pct[:],
                                scalar1=recip_s[:, h, dt_:dt_ + 1])

                    # out-proj + residual + LN1 + LN2
                    out2_s = poolB.tile([128, NCH, D], f32, tag="out2", bufs=1)
                    for st in range(NCH):
                        pwo = psB.tile([128, D], f32, tag="pwo", bufs=1)
                        for kt in range(NST):
                            nc.tensor.matmul(
                                pwo[:], ctx_s[:, kt, st * 128:(st + 1) * 128],
                                wo_s[:, kt, :],
                                start=(kt == 0), stop=(kt == NST - 1))
                        gst = chk * NCH + st
                        t1 = poolB.tile([128, D], f32, tag="t1", bufs=2)
                        nc.vector.tensor_add(out=t1[:], in0=pwo[:], in1=x_s[:, gst, :])
                        o1 = poolB.tile([128, D], f32, tag="o1", bufs=2)
                        ln_block(poolB, t1[:], "g1b", "be1b", o1[:])
                        nc.vector.tensor_add(out=o1[:], in0=o1[:], in1=x_s[:, gst, :])
                        ln_block(poolB, o1[:], "g2b", "be2b", out2_s[:, st, :])

                    # transpose out2 chunk -> [c, s]
                    o2t = poolB.tile([128, NCT, CH], f32r, tag="o2t", bufs=1)
                    for st in range(NCH):
                        for ct in range(NCT):
                            ptr = psB.tile([128, 128], f32, tag="ptr", bufs=1)
                            nc.tensor.transpose(
                                ptr[:], out2_s[:, st, ct * 128:(ct + 1) * 128], ident_s[:])
                            nc.vector.tensor_copy(
                                out=o2t[:, ct, st * 128:(st + 1) * 128], in_=ptr[:])

                    # FFN1 (+bias +relu), h1^T [j, s]
                    h1_s = poolB.tile([128, NST, CH], f32r, tag="h1", bufs=1)
                    for jt in range(NST):
                        ph1 = psB.tile([128, CH], f32, tag="ph1", bufs=2)
                        for kt in range(NCT):
                            nc.tensor.matmul(
                                ph1[:], w1_s[:, kt, jt * 128:(jt + 1) * 128],
                                o2t[:, kt, :],
                                start=(kt == 0), stop=(kt == NCT - 1))
                        nc.scalar.activation(out=h1_s[:, jt, :], in_=ph1[:],
                                             func=AF.Relu, bias=b1t_s[:, jt:jt + 1],
                                             scale=1.0)

                    # FFN2 + bias + residual + LN3 -> DMA
                    for st in range(NCH):
                        pf2 = psB.tile([128, D], f32, tag="pf2", bufs=1)
                        for jt in range(NST):
                            nc.tensor.matmul(
                                pf2[:], h1_s[:, jt, st * 128:(st + 1) * 128],
                                w2_s[:, jt, :],
                                start=(jt == 0), stop=(jt == NST - 1))
                        t2 = poolB.tile([128, D], f32, tag="t2", bufs=2)
                        nc.vector.tensor_add(out=t2[:], in0=pf2[:], in1=out2_s[:, st, :])
                        nc.vector.tensor_add(out=t2[:], in0=t2[:], in1=bvec_s["b2b"][:])
                        oo = poolB.tile([128, D], f32, tag="oo", bufs=2)
                        ln_block(poolB, t2[:], "g3b", "be3b", oo[:])
                        gst = chk * NCH + st
                        nc.sync.dma_start(
                            out=out_d[gst * 128:(gst + 1) * 128, :], in_=oo[:])

    import concourse.mybir as mybir2
    _split_multi_waits(nc, mybir2)
    return nc


def _host_prep(inputs):
    x = np.asarray(inputs["enc_inputs"], dtype=np.float32)        # [B, S, D]
    A = np.asarray(inputs["A"], dtype=np.float32)                 # [D, D]
    Wk = np.asarray(inputs["Wk"], dtype=np.float32)               # [D, HID]
    Wq = np.asarray(inputs["Wq"], dtype=np.float32)
    scale = np.float32(1.0 / np.sqrt(D))
    # fold A into K/Q projections (per head)
    WKP = np.einsum("chd,de->che", Wk.reshape(D, H, D), A).reshape(D, HID) * scale
    WQP = np.einsum("chd,de->che", Wq.reshape(D, H, D), A).reshape(D, HID)
    b1 = np.asarray(inputs["b1"], dtype=np.float32)
    b2 = np.asarray(inputs["b2"], dtype=np.float32)

    def bcast(v):
        return np.ascontiguousarray(
            np.broadcast_to(np.asarray(v, np.float32)[None, :], (128, D)))

    common = {
        "wkp": np.ascontiguousarray(WKP),
        "wqp": np.ascontiguousarray(WQP),
        "wv": np.ascontiguousarray(inputs["Wv"], dtype=np.float32),
        "wo": np.ascontiguousarray(inputs["Wo"], dtype=np.float32),
        "w1": np.ascontiguousarray(inputs["W1"], dtype=np.float32),
        "w2": np.ascontiguousarray(inputs["W2"], dtype=np.float32),
        "b1t": np.ascontiguousarray(b1.reshape(NST, 128).T),
        "b2b": bcast(b2),
        "g1b": bcast(inputs["ln_attn_g"]), "be1b": bcast(inputs["ln_attn_b"]),
        "g2b": bcast(inputs["ln_enc_g"]), "be2b": bcast(inputs["ln_enc_b"]),
        "g3b": bcast(inputs["ln_ffn_g"]), "be3b": bcast(inputs["ln_ffn_b"]),
    }
    in_maps = []
    for b in range(B):
        m = dict(common)
        m["x"] = np.ascontiguousarray(x[b])
        m["xT"] = np.ascontiguousarray(x[b].T)
        in_maps.append(m)
    return in_maps


def kernel(**inputs):
    import os
    from concourse.bass_utils import run_bass_kernel_spmd

    if "nc" not in _CACHE:
        _CACHE["nc"] = _build()
    nc = _CACHE["nc"]
    in_maps = _host_prep(inputs)
    trace = bool(os.environ.get("KERNEL_TRACE"))
    res = run_bass_kernel_spmd(nc, in_maps, list(range(B)), trace=trace,
                               tmpdir=os.environ.get("KERNEL_TRACE_DIR") or None)
    if trace:
        _CACHE["last_result"] = res
        if res.exec_time_ns is not None:
            print(f"HW exec time: {res.exec_time_ns} ns")
    out = np.stack([res.results[b]["out_o"] for b in range(B)])      # [B, S, D]
    attn = np.stack([res.results[b]["attn_o"] for b in range(B)])    # [B, H, D, S]
    return out, attn
